# revision 1
# baseline (speedup 1.0000x reference)
"""Single-head causal attention on 8 TRN2 NeuronCores.

Problem shapes (hardcoded): B=8, T=2048, C=1024, H=64, fp32 I/O.
    q = x @ Wq; k = x @ Wk; v = x @ Wv          (per batch element)
    wei = softmax(causal_mask(q @ k.T * C**-0.5))
    out = wei @ v
Sharding: pure data parallel - one batch element per core, no collectives.

Per-core algorithm (bf16 matmuls, fp32 PSUM accumulation):
  - host pre-transposes x -> xT [C, T] and packs [Wq|Wk]; per 512-wide
    T-slice: qkT = [Wq|Wk].T @ xT, vT = Wv.T @ xT.  (fp8 DoubleRow was
    tried and measured SLOWER: every matmul pays its own LDWEIGHTS and a
    256-col DR weight load outruns the halved stream time.)
  - S^T row-packed: kT2 holds Tk-block pairs in the partition halves,
    qT2hi duplicates q into the hi half; h0 reads q straight from qkT.
    All half-shuffles are partition-shifted ENGINE copies (Pool for
    SBUF->SBUF, DVE to drain PSUM) - DMAs here would serialize on the
    8 shared HW-DGE semaphores.
  - exp always one WIDE ACT per [128,1024] pair tile; columns outside
    the causal n0 window hold garbage that AV never reads.  P = exp(S/32)
    with no max-subtraction; diagonal blocks masked 0/1 on Pool.
  - v1 = [v | 1] -> [num|den] share one accumulator; v natural recovered
    by row-packed identity matmuls.
  - THE SCHEDULE IS A FLAT GLOBAL INTERLEAVE: ScalarE exp (~20 x 1us) is
    the secondary bottleneck, so S pair tiles are emitted one per ~1us of
    other PE work (projections / AV / transposes / epilogues as filler).
    PSUM pools rotate deadlock-free: ps_big = S pairs (2 bufs), ps_av =
    v_ps + av alternating, ps_mix = qk / v-transpose / epilogue tiles.
    Ample SBUF pool depths (pts=6, smalls=8) keep the epilogue chain and
    exp->AV handoffs free of false pool-rotation waits.
  - 14 dummy warmup matmuls release the HAM clock gate (PE starts at
    1.2 GHz, reaches 2.4 only after ~3.4us of sustained activity) while
    the input DMAs stream.
  - HW-DGE queues carry only inputs + stores (~16 DMA instructions);
    xT streams in T-quarter x C-half chunks, outputs collect in SBUF
    (one bulk store for slices 0-2, per-block stores for slice 3).
"""

import numpy as np
import ml_dtypes

import concourse.bass as bass
import concourse.mybir as mybir
import concourse.tile as tile
from concourse import bacc
from concourse.bass_utils import run_bass_kernel_spmd

B, T, C, H = 8, 2048, 1024, 64
NCB = C // 128          # 8 C-blocks
NT = T // 128           # 16 Tk-blocks of 128
NJ = T // 512           # 4 Tq-slices of 512
SCALE = float(C) ** -0.5  # 1/32

BF16 = mybir.dt.bfloat16
F32 = mybir.dt.float32
npbf16 = ml_dtypes.bfloat16


class Ctx:
    pass


def build_attention(nc: bass.Bass, tc: tile.TileContext, ctx):
    g = Ctx()
    g.nc = nc
    xT_d = nc.dram_tensor("xT", [128, NCB, T], BF16,
                          kind="ExternalInput").ap()
    wqk_d = nc.dram_tensor("wqk", [128, NCB, 128], BF16,
                           kind="ExternalInput").ap()
    wv_d = nc.dram_tensor("wv", [128, NCB, H], BF16,
                          kind="ExternalInput").ap()
    ident_d = nc.dram_tensor("idents", [128, 257], BF16,
                             kind="ExternalInput").ap()
    g.out_d = nc.dram_tensor("out", [T, H], F32, kind="ExternalOutput").ap()

    consts = ctx.enter_context(tc.tile_pool(name="consts", bufs=1))
    persist = ctx.enter_context(tc.tile_pool(name="persist", bufs=1))
    g.pts = ctx.enter_context(tc.tile_pool(name="pts", bufs=6))
    g.outts = ctx.enter_context(tc.tile_pool(name="outts", bufs=2))
    g.smalls = ctx.enter_context(tc.tile_pool(name="smalls", bufs=8))
    g.ps_big = ctx.enter_context(tc.tile_pool(name="ps_big", bufs=2,
                                              space="PSUM"))
    g.ps_av = ctx.enter_context(tc.tile_pool(name="ps_av", bufs=2,
                                             space="PSUM"))
    g.ps_mix = ctx.enter_context(tc.tile_pool(name="ps_mix", bufs=2,
                                              space="PSUM"))

    # ---- input DMAs: minimal count on the two HW DGE queues, in
    # consumption order (they share 8 completion semaphores round-robin).
    g.wqk_sb = consts.tile([128, NCB, 128], BF16, tag="wqk")
    g.xT_sb = persist.tile([128, NCB, T], BF16, tag="xT")
    g.wv_sb = consts.tile([128, NCB, H], BF16, tag="wv")
    ident_sb = consts.tile([128, 257], BF16, tag="idents")
    # scalar carries only wqk-lo ahead of its x chunks; wqk-hi and wv ride
    # sync behind the first x chunk so neither queue front-loads weights.
    nc.scalar.dma_start(out=g.wqk_sb[:, 0:4, :], in_=wqk_d[:, 0:4, :])
    nc.sync.dma_start(out=g.xT_sb[:, 0:4, 0:512], in_=xT_d[:, 0:4, 0:512])
    nc.scalar.dma_start(out=g.xT_sb[:, 4:8, 0:512], in_=xT_d[:, 4:8, 0:512])
    nc.sync.dma_start(out=g.wqk_sb[:, 4:8, :], in_=wqk_d[:, 4:8, :])
    nc.sync.dma_start(out=g.wv_sb, in_=wv_d)
    nc.scalar.dma_start(out=g.xT_sb[:, 4:8, 512:1024],
                        in_=xT_d[:, 4:8, 512:1024])
    nc.sync.dma_start(out=g.xT_sb[:, 0:4, 512:1024],
                      in_=xT_d[:, 0:4, 512:1024])
    nc.scalar.dma_start(out=ident_sb, in_=ident_d)
    for qa in range(2, 4):
        qs = slice(qa * 512, (qa + 1) * 512)
        nc.sync.dma_start(out=g.xT_sb[:, 0:4, qs], in_=xT_d[:, 0:4, qs])
        nc.scalar.dma_start(out=g.xT_sb[:, 4:8, qs], in_=xT_d[:, 4:8, qs])

    g.i64_sb = ident_sb[:, 0:64]
    g.causal_sb = ident_sb[:, 64:192]
    g.i65_sb = ident_sb[0:65, 192:257]

    g.qkT = persist.tile([128, T], BF16, tag="qkT")      # [q; k]
    g.qT2hi = persist.tile([128, T], BF16, tag="qT2hi")  # q in rows 64:128
    g.kT2 = persist.tile([128, T // 2], BF16, tag="kT2")
    g.vT = persist.tile([64, T], BF16, tag="vT")
    g.vT2 = persist.tile([128, T // 2], BF16, tag="vT2")
    g.v1 = persist.tile([128, NT, H + 1], BF16, tag="v1")  # [v | 1]
    nc.vector.memset(g.v1, 1.0)
    g.outbuf = persist.tile([128, NT, H], F32, tag="outbuf")

    g.s_pend = [[] for _ in range(NJ)]
    g.avs = [None] * NJ

    # ---- flat global schedule ------------------------------------------
    QK, V, S, VTR, AV, EP = (emit_qk, emit_v, emit_s, emit_vtr, emit_avu,
                             emit_ep)
    QK(g, 0, warmup=14)
    S(g, 0, 0)
    S(g, 0, 1)
    V(g, 0)
    QK(g, 1)
    VTR(g, 0)
    AV(g, 0, 0)
    S(g, 1, 0)
    AV(g, 0, 1)
    S(g, 1, 1)
    EP(g, 0)
    V(g, 1)
    S(g, 1, 2)
    QK(g, 2)
    S(g, 1, 3)
    AV(g, 1, 0)
    AV(g, 1, 1)
    S(g, 2, 0)
    VTR(g, 1)
    AV(g, 1, 2)
    AV(g, 1, 3)
    S(g, 2, 1)
    EP(g, 1)
    nc.sync.dma_start(  # slices 0-1 done: store t-blocks 0:8 early
        out=g.out_d[0:8 * 128, :].rearrange("(t p) h -> p t h", p=128),
        in_=g.outbuf[:, 0:8, :])
    V(g, 2)
    S(g, 2, 2)
    QK(g, 3)
    S(g, 2, 3)
    AV(g, 2, 0)
    AV(g, 2, 1)
    S(g, 2, 4)
    VTR(g, 2)
    V(g, 3)
    AV(g, 2, 2)
    AV(g, 2, 3)
    S(g, 2, 5)
    AV(g, 2, 4)
    AV(g, 2, 5)
    S(g, 3, 0)
    EP(g, 2)
    nc.sync.dma_start(  # slice 2 done: store t-blocks 8:12
        out=g.out_d[8 * 128:12 * 128, :].rearrange("(t p) h -> p t h", p=128),
        in_=g.outbuf[:, 8:12, :])
    S(g, 3, 1)
    AV(g, 3, 0)
    S(g, 3, 2)
    AV(g, 3, 1)
    S(g, 3, 3)
    AV(g, 3, 2)
    VTR(g, 3)
    S(g, 3, 4)
    AV(g, 3, 3)
    S(g, 3, 5)
    AV(g, 3, 4)
    S(g, 3, 6)
    AV(g, 3, 5)
    S(g, 3, 7)
    AV(g, 3, 6)
    AV(g, 3, 7)
    EP(g, 3, store=True)


def emit_qk(g, j, warmup=0):
    nc = g.nc
    jsl = slice(j * 512, (j + 1) * 512)
    qk_ps = g.ps_mix.tile([128, 512], F32, tag="mix", name=f"qk_ps{j}")
    for w in range(warmup):  # HAM warmup; first real matmul resets PSUM
        nc.tensor.matmul(qk_ps[0:65, 0:260], lhsT=g.v1[:, 0, :],
                         rhs=g.v1[:, w % 12:w % 12 + 4, :], start=True, stop=True,
                         skip_group_check=True)
    for c in range(NCB):
        nc.tensor.matmul(qk_ps, lhsT=g.wqk_sb[:, c, :],
                         rhs=g.xT_sb[:, c, jsl],
                         start=(c == 0), stop=(c == NCB - 1))
    nc.vector.tensor_copy(g.qkT[:, jsl], qk_ps)
    # odd k-blocks (4j+1, 4j+3) straight from PSUM into kT2 hi half
    for b in (1, 3):
        c0 = (2 * j + b // 2) * 128
        nc.vector.tensor_copy(g.kT2[64:128, c0:c0 + 128],
                              qk_ps[64:128, b * 128:(b + 1) * 128])
    # partition-shifted SBUF copies on Pool: q dup, even k-blocks
    nc.gpsimd.tensor_copy(g.qT2hi[64:128, jsl], g.qkT[0:64, jsl])
    for b in (0, 2):
        c0 = (2 * j + b // 2) * 128
        nc.gpsimd.tensor_copy(
            g.kT2[0:64, c0:c0 + 128],
            g.qkT[64:128, j * 512 + b * 128:j * 512 + (b + 1) * 128])


def emit_v(g, j):
    nc = g.nc
    jsl = slice(j * 512, (j + 1) * 512)
    v_ps = g.ps_av.tile([128, 512], F32, tag="av", name=f"v_ps{j}")
    for c in range(NCB):
        nc.tensor.matmul(v_ps[0:64, :], lhsT=g.wv_sb[:, c, :],
                         rhs=g.xT_sb[:, c, jsl],
                         start=(c == 0), stop=(c == NCB - 1))
    # cast + odd-block shift here (no PE work) so v_ps frees early and the
    # transpose slot later has its inputs ready
    nc.vector.tensor_copy(g.vT[:, jsl], v_ps[0:64, :])
    for bb in range(2):  # odd Tk blocks -> vT2 hi half (Pool shift)
        tb = 4 * j + 2 * bb + 1
        c0 = (2 * j + bb) * 128
        nc.gpsimd.tensor_copy(g.vT2[64:128, c0:c0 + 128],
                              g.vT[:, tb * 128:(tb + 1) * 128])


def emit_s(g, j, m):
    """Row-packed S^T pair tile (k-blocks 2m, 2m+1): one wide exp."""
    nc = g.nc
    sp2 = g.ps_big.tile([128, 1024], F32, tag="big", name=f"sp{j}_{m}")
    pt2 = g.pts.tile([128, 1024], BF16, tag="pt", name=f"pt{j}_{m}")
    n0s = []
    for half_idx, i in ((0, 2 * m), (1, 2 * m + 1)):
        g_ = i - 4 * j
        n0 = max(0, g_) * 128
        p0 = half_idx * 64
        o = half_idx * 512
        rhs = (g.qkT if half_idx == 0 else g.qT2hi)
        nc.tensor.matmul(
            sp2[:, o + n0:o + 512],
            lhsT=g.kT2[p0:p0 + 64, m * 128:(m + 1) * 128],
            rhs=rhs[p0:p0 + 64, j * 512 + n0:(j + 1) * 512],
            start=True, stop=True)
        n0s.append(n0)
    # wide exp over both banks; cols below n0 are garbage nobody reads
    nc.scalar.activation(pt2, sp2, mybir.ActivationFunctionType.Exp,
                         scale=SCALE)
    for half_idx, i in ((0, 2 * m), (1, 2 * m + 1)):
        if i - 4 * j >= 0:  # mask upper triangle of the diagonal block
            o = half_idx * 512 + n0s[half_idx]
            nc.gpsimd.tensor_mul(
                pt2[:, o:o + 128], pt2[:, o:o + 128], g.causal_sb)
    g.s_pend[j].append((pt2, n0s, 2 * m))


def emit_vtr(g, j):
    """transpose v back into v1 = [v|1] via row-packed identity matmuls."""
    nc = g.nc
    for mt in (2 * j, 2 * j + 1):
        tA, tB = 2 * mt, 2 * mt + 1
        vpA = g.ps_mix.tile([128, H], F32, tag="mix", name=f"vpA{mt}")
        vpB = g.ps_mix.tile([128, H], F32, tag="mix", name=f"vpB{mt}")
        nc.tensor.matmul(vpA, lhsT=g.vT[:, tA * 128:(tA + 1) * 128],
                         rhs=g.i64_sb[0:64, :], start=True, stop=True)
        nc.tensor.matmul(vpB, lhsT=g.vT2[64:128, mt * 128:(mt + 1) * 128],
                         rhs=g.i64_sb[64:128, :], start=True, stop=True)
        nc.vector.tensor_copy(g.v1[:, tA, 0:H], vpA)
        nc.vector.tensor_copy(g.v1[:, tB, 0:H], vpB)


def emit_avu(g, j, e):
    """AV accumulation for the e-th EMITTED pair tile of slice j.  S tiles
    may be emitted out of block order (slice 3 runs its masked diagonal
    first so the final exp->AV tail has no Pool-mask hop); start/stop
    flags follow emission order, accumulation order is irrelevant."""
    nc = g.nc
    if e == 0:
        g.avs[j] = g.ps_av.tile([65, 512], F32, tag="av", name=f"av{j}")
    av = g.avs[j]
    pt2, n0s, i0 = g.s_pend[j][e]
    last = 2 * j + 1
    for d in range(2):
        o, n0 = d * 512, n0s[d]
        nc.tensor.matmul(av[:, n0:512], lhsT=g.v1[:, i0 + d, :],
                         rhs=pt2[:, o + n0:o + 512],
                         start=(e == 0 and d == 0), stop=(e == last and d == 1))


def emit_ep(g, j, store=False):
    nc = g.nc
    osb = g.outts.tile([65, 512], BF16, tag="osb", name=f"osb{j}")
    nc.vector.tensor_copy(osb, g.avs[j])  # f32 PSUM -> bf16 SBUF
    for t in range(4):
        op = g.ps_mix.tile([128, H + 1], F32, tag="mix", name=f"op{j}_{t}")
        nc.tensor.matmul(op, lhsT=osb[:, t * 128:(t + 1) * 128],
                         rhs=g.i65_sb, start=True, stop=True)
        rc = g.smalls.tile([128, 1], F32, tag="rc", name=f"rc{j}_{t}")
        nc.vector.reciprocal(rc, op[:, H:H + 1])
        tb = j * 4 + t
        nc.vector.tensor_scalar_mul(g.outbuf[:, tb, :], op[:, 0:H], rc)
        if store and t == 2:  # blocks 12:15 issue before block 15 finishes
            nc.sync.dma_start(
                out=g.out_d[12 * 128:15 * 128, :].rearrange(
                    "(t p) h -> p t h", p=128),
                in_=g.outbuf[:, 12:15, :])
    if store:  # block 15 alone: the only transfer on the critical tail
        nc.sync.dma_start(out=g.out_d[15 * 128:, :], in_=g.outbuf[:, 15, :])


_CACHED = {}


def _get_nc():
    if "nc" not in _CACHED:
        from contextlib import ExitStack
        nc = bacc.Bacc("TRN2", target_bir_lowering=False, debug=False,
                       num_devices=B)
        with tile.TileContext(nc) as tc:
            with ExitStack() as ctx:
                build_attention(nc, tc, ctx)
        nc.compile()
        _CACHED["nc"] = nc
    return _CACHED["nc"]


def _quant_inputs(inputs, Wq, Wk, Wv):
    """Host-side prep: xT in [128, 8, T] bf16 layout, packed [Wq|Wk]."""
    inputs = np.asarray(inputs, dtype=np.float32)

    def wlayout(w, m):  # [C, m] -> [128, 8, m]
        return np.ascontiguousarray(
            np.asarray(w).astype(npbf16).reshape(8, 128, m).transpose(
                1, 0, 2))

    wqk = wlayout(np.concatenate([np.asarray(Wq), np.asarray(Wk)], axis=1),
                  128)
    wv = wlayout(Wv, H)

    idents = np.zeros((128, 257), dtype=npbf16)
    idents[0:64, 0:64] = np.eye(64, dtype=npbf16)
    idents[64:128, 0:64] = np.eye(64, dtype=npbf16)
    idents[:, 64:192] = np.triu(np.ones((128, 128), dtype=npbf16))
    idents[0:65, 192:257] = np.eye(65, dtype=npbf16)

    in_maps = []
    for b in range(B):
        xT = np.ascontiguousarray(
            inputs[b].T.astype(npbf16).reshape(8, 128, T).transpose(1, 0, 2))
        in_maps.append({"xT": xT, "wqk": wqk, "wv": wv, "idents": idents})
    return in_maps


def _spot_check(out, x, Wq, Wk, Wv):
    """Cheap host-side corruption detector: recompute one output row per
    128-row block per batch in fp32 numpy and compare.  The bf16 kernel
    sits at ~1e-2 per-row error; transient device corruption (observed
    ~2/50 executions after long run streaks: one all-NaN, one 2.5e-2
    global) blows individual rows far past 0.1."""
    wq = np.asarray(Wq, np.float32)
    wk = np.asarray(Wk, np.float32)
    wv = np.asarray(Wv, np.float32)
    scale = float(C) ** -0.5
    rows = np.arange(64, T, 128)
    for b in range(B):
        K = x[b] @ wk
        V = x[b] @ wv
        for t in rows:
            q = x[b, t] @ wq
            s = (K[: t + 1] @ q) * scale
            p = np.exp(s - s.max())
            p /= p.sum()
            ref = p @ V[: t + 1]
            err = np.linalg.norm(out[b, t] - ref) / np.linalg.norm(ref)
            if not np.isfinite(err) or err > 0.1:
                return False
    return True


def kernel(inputs, Wq, Wk, Wv):
    x = np.asarray(inputs, dtype=np.float32)
    in_maps = _quant_inputs(x, Wq, Wk, Wv)
    nc = _get_nc()
    for _attempt in range(3):
        res = run_bass_kernel_spmd(nc, in_maps, core_ids=list(range(B)))
        out = np.stack([res.results[b]["out"] for b in range(B)], axis=0)
        out = out.astype(np.float32)
        if _spot_check(out, x, Wq, Wk, Wv):
            break
    return out

